# revision 2
# baseline (speedup 1.0000x reference)
"""Bass/Trainium2 kernel v3 for nn_DecoderModel (B=4 T=1024 D=1024 H=16 L=12 V=50257).

Sharding: 8 cores; core c = (batch b=c//2, parity p=c%2). Parity p owns the
4 global 128-token q-tiles {2j+1-p}. Residual stream transposed in SBUF as
[128, 8, 512] (d-partition, d-tile, token).

v3 changes vs v2:
- AllGather ships the LN1-normed x (1MB) instead of K/V (2.1MB); each core
  computes the peer block's K/V locally from the gathered x. Halves the
  collective cost and fully hides it under own-token K/V/Q + local attention.
- Softmax denominator of the even head rides in the AV matmul (lhs [V|1],
  M=65); only the odd head keeps a ones-matmul denominator.
- All broadcast matmuls (LN mean/rstd, attention 1/den) run as fp32r
  (1 cycle/row at N=512 vs 4 for fp32).
- LN: stats matmuls read x directly as fp32r (no bf16 copy of x); the
  normalize is 2 DVE ops per d-tile with scale/bias folded via a K=2
  broadcast matmul (host packs ones/-b/s rows).
- exp for both heads of a pair in one ACT instruction ([128,2,512] PSUM);
  diag-mask multiply for both heads in one gpsimd op.
- lm_b applied on the host during assembly (removes 4 bias matmuls per
  vocab group); per-layer scales/biases arrive in one packed DMA.
"""
import os
import sys

sys.path.insert(0, "/opt/trn_rl_repo")

import numpy as np
import ml_dtypes

import concourse.bass as bass
import concourse.mybir as mybir
import concourse.tile as tile
from concourse import bacc
from concourse.bass_utils import run_bass_kernel_spmd

BF16 = mybir.dt.bfloat16
F32 = mybir.dt.float32
F32R = mybir.dt.float32r
FP8 = mybir.dt.float8e4

B, T, D, H, NL_FULL, V = 4, 1024, 1024, 16, 12, 50257
DH = D // H              # 64
DT = D // 128            # 8 d-tiles
QT = 4                   # q-tiles (128 rows) per core
VPAD = 50688             # 99 * 512
NVG = VPAD // 512        # 99 vocab groups
LN_EPS = 1e-5
INV_SQRT_C = 1.0 / 32.0

L = int(os.environ.get("BASSK_L", str(NL_FULL)))
PHASE_MARKS = []

AG_IN_ELEMS = 128 * DT * 512       # normed-x block: [p, dt, tok] bf16
AG_OUT_ELEMS = 2 * AG_IN_ELEMS

# packed per-layer params: columns [s1 8][s2 8][bo 8][b1 32][b2 8]
PP_S1, PP_S2, PP_BO, PP_B1, PP_B2, PP_W = 0, 8, 16, 24, 56, 64


def build_nc(num_layers=L):
    PHASE_MARKS.clear()
    nc = bacc.Bacc("TRN2", target_bir_lowering=False, debug=True)
    NL = num_layers

    def mark(name):
        PHASE_MARKS.append((name, int(nc.next_id())))

    x0T = nc.declare_dram_parameter("x0T", [128, DT, 512], F32, isOutput=False)
    wqp = nc.declare_dram_parameter("wqp", [NL, 128, DT, DT, 128], BF16, isOutput=False)
    wkp = nc.declare_dram_parameter("wkp", [NL, 128, DT, DT, 128], BF16, isOutput=False)
    wvp = nc.declare_dram_parameter("wvp", [NL, 128, 2, DT, 512], BF16, isOutput=False)
    wop = nc.declare_dram_parameter("wop", [NL, 128, DT, DT, 128], BF16, isOutput=False)
    w1p = nc.declare_dram_parameter("w1p", [NL, 128, 32, DT, 128], BF16, isOutput=False)
    w2p = nc.declare_dram_parameter("w2p", [NL, 128, DT, 32, 128], BF16, isOutput=False)
    pp = nc.declare_dram_parameter("pp", [NL, 128, PP_W], F32, isOutput=False)
    # LN broadcast -b/s rows (partition 0): [l, 1, ln(2), dt, dcol]
    lnlhs = nc.declare_dram_parameter("lnlhs", [NL, 1, 2, DT, 128], BF16,
                                      isOutput=False)
    lnfs = nc.declare_dram_parameter("lnfs", [128, DT], F32, isOutput=False)
    lnflhs = nc.declare_dram_parameter("lnflhs", [1, 2, DT, 128], BF16,
                                       isOutput=False)
    lmwp = nc.declare_dram_parameter("lmwp", [NVG, 128, DT, 512], BF16,
                                     isOutput=False)
    # mident[1] = local diag mult-mask (tril); mident[2] = remote diag
    # mult-mask (ones for parity 0, zeros for parity 1); [0] unused pad.
    mident = nc.declare_dram_parameter("mident", [3, 128, 2, 128], BF16,
                                       isOutput=False)
    peer_i = nc.declare_dram_parameter("peer_i", [1, 1], mybir.dt.int32,
                                       isOutput=False)
    out = nc.declare_dram_parameter("out", [512, VPAD], BF16, isOutput=True)

    ag_in = [nc.dram_tensor(f"ag_in{i}", [AG_IN_ELEMS], FP8) for i in range(2)]
    ag_out = [nc.dram_tensor(f"ag_out{i}", [AG_OUT_ELEMS], FP8) for i in range(2)]
    groups = [[0, 1], [2, 3], [4, 5], [6, 7]]

    from contextlib import ExitStack
    with tile.TileContext(nc) as tc, ExitStack() as es:
        const = es.enter_context(tc.tile_pool(name="const", bufs=1))
        xpool = es.enter_context(tc.tile_pool(name="xpool", bufs=1))
        npool = es.enter_context(tc.tile_pool(name="npool", bufs=1))
        small = es.enter_context(tc.tile_pool(name="small", bufs=1))

        ones_bf = const.tile([128, 1], BF16)
        nc.vector.memset(ones_bf[:], 1.0)
        onesD_bf = const.tile([128, 1], BF16)
        nc.vector.memset(onesD_bf[:], 1.0 / D)
        onesb = const.tile([128, 128], BF16)
        nc.vector.memset(onesb[:], 1.0)
        eps_t = const.tile([1, 1], F32)
        nc.vector.memset(eps_t[:], LN_EPS)
        mask_t = const.tile([128, 3, 2, 128], BF16)
        nc.sync.dma_start(mask_t[:], mident.rearrange("m k d q -> k m d q"))
        lnf_s_t = const.tile([128, DT], F32)
        nc.sync.dma_start(lnf_s_t[:], lnfs[:])
        lnf_lhs_t = const.tile([1, 2, DT, 128], BF16)
        nc.sync.dma_start(lnf_lhs_t[:], lnflhs[:])

        xT = xpool.tile([128, DT, 512], F32, name="xT", tag="x")
        nc.sync.dma_start(xT[:], x0T[:])

        # peer block index for the gathered buffer (register on gpsimd)
        peer_reg = nc.gpsimd.alloc_register("peer_reg")
        nc.gpsimd.reg_load(peer_reg, peer_i[0:1, 0:1])
        peer = nc.gpsimd.snap(peer_reg, donate=True, min_val=0, max_val=1)

        def layernorm(x_in, s_ap, lhs_ap, fp8_out=None):
            """x_in [128, DT, 512] f32 -> nbf [128, DT, 512] bf16.

            s_ap: [128, DT] f32 SBUF (per-d scale s)
            lhs_ap: [1, DT, 128] f32 SBUF (-b/s row at partition 0)
            nbf = (x - (mu + (-b/s)*sd)) * rstd * s
            fp8_out: optional [128, DT, 512] fp8 tile for an extra copy.
            """
            nbf = npool.tile([128, DT, 512], BF16, name="nbf", tag="nbf")
            with tc.tile_pool(name="lnp", bufs=1, space="PSUM") as lnp, \
                 tc.tile_pool(name="lns", bufs=1) as lns:
                ps1 = lnp.tile([1, 512], F32, name="ps1", tag="ps1")
                ps2 = lnp.tile([1, 512], F32, name="ps2", tag="ps2")
                for dt_i in range(DT):
                    sq = lns.tile([128, 512], BF16, name="sq", tag="sq",
                                  bufs=2)
                    nc.scalar.square(sq[:], x_in[:, dt_i])
                    xbf = lns.tile([128, 512], BF16, name="xbf", tag="xbf",
                                   bufs=2)
                    nc.vector.tensor_copy(xbf[:], x_in[:, dt_i])
                    nc.tensor.matmul(ps1[:], onesD_bf[:], xbf[:],
                                     start=(dt_i == 0), stop=(dt_i == DT - 1))
                    nc.tensor.matmul(ps2[:], onesD_bf[:], sq[:],
                                     start=(dt_i == 0), stop=(dt_i == DT - 1))
                mu_row = lns.tile([1, 512], BF16, name="mu_row", tag="mu_row")
                with nc.allow_low_precision(reason="bf16 LN stat rows"):
                    nc.vector.tensor_copy(mu_row[:], ps1[:])
                musq = lns.tile([1, 512], F32, name="musq", tag="musq")
                nc.scalar.square(musq[:], ps1[:])
                nc.vector.tensor_sub(musq[:], ps2[:], musq[:])
                sd_row = lns.tile([1, 512], BF16, name="sd_row", tag="sd_row")
                nc.scalar.activation(sd_row[:], musq[:],
                                     mybir.ActivationFunctionType.Sqrt,
                                     bias=eps_t[:])
                rstd_bf = lns.tile([1, 512], BF16, name="rstd_bf",
                                   tag="rstd_bf")
                with nc.allow_low_precision(reason="bf16 LN rstd row"):
                    nc.vector.reciprocal(rstd_bf[:], sd_row[:])
                rstdb = lnp.tile([128, 512], F32, name="rstdb", tag="rstdb")
                nc.tensor.matmul(rstdb[:], onesb[0:1, :], rstd_bf[:],
                                 start=True, stop=True)
                for dt_i in range(DT):
                    mub2 = lnp.tile([128, 512], F32, name="mub2", tag="mub2",
                                    bufs=2)
                    nc.tensor.matmul(mub2[:], onesb[0:1, :], mu_row[:],
                                     start=True, stop=False)
                    nc.tensor.matmul(mub2[:], lhs_ap[:, dt_i, :],
                                     sd_row[:], start=False, stop=True)
                    t1 = lns.tile([128, 512], F32, name="t1", tag="t1",
                                  bufs=2)
                    nc.vector.tensor_sub(t1[:], x_in[:, dt_i], mub2[:])
                    nc.vector.scalar_tensor_tensor(
                        nbf[:, dt_i], t1[:], s_ap[:, dt_i:dt_i + 1], rstdb[:],
                        mybir.AluOpType.mult, mybir.AluOpType.mult)
                    if fp8_out is not None:
                        if dt_i % 2 == 0:
                            nc.scalar.copy(fp8_out[:, dt_i], nbf[:, dt_i])
                        else:
                            nc.vector.tensor_copy(fp8_out[:, dt_i],
                                                  nbf[:, dt_i])
            return nbf

        with ExitStack() as les:
            proj = les.enter_context(tc.tile_pool(name="proj", bufs=1))
            wpool = les.enter_context(tc.tile_pool(name="wpool", bufs=2))
            locp = les.enter_context(tc.tile_pool(name="locp", bufs=1))
            stp = les.enter_context(tc.tile_pool(name="stp", bufs=2))
            sump = les.enter_context(tc.tile_pool(name="sump", bufs=1))
            peerp = les.enter_context(tc.tile_pool(name="peerp", bufs=1))

            def proj_kt(n_src, w_dram, out_tag):
                """K^T/Q^T-style projection: [128, DT, 512] from n_src."""
                kt = proj.tile([128, DT, 512], BF16, name=out_tag,
                               tag=out_tag)
                with tc.tile_pool(name="pk", bufs=1, space="PSUM") as pk:
                    for half in range(2):
                        w_t = wpool.tile([128, 4, DT, 128], BF16, name="w_t",
                                         tag="wbig")
                        nc.sync.dma_start(w_t[:],
                                          w_dram[:, half * 4:(half + 1) * 4])
                        for fi in range(4):
                            ft = half * 4 + fi
                            pq = pk.tile([128, 512], F32, name="pq", tag="pq",
                                         bufs=3)
                            for dt_i in range(DT):
                                nc.tensor.matmul(pq[:], w_t[:, fi, dt_i],
                                                 n_src[:, dt_i],
                                                 start=(dt_i == 0),
                                                 stop=(dt_i == DT - 1))
                            if ft % 2 == 0:
                                nc.scalar.copy(kt[:, ft], pq[:])
                            else:
                                nc.vector.tensor_copy(kt[:, ft], pq[:])
                return kt

            def proj_v(n_src, w_dram, out_tag):
                """V projection: [128(tok), H, 4(tok-tile), 96].

                Even heads (A): cols [0:64] = v, col 64 = ones (denominator
                rides the AV matmul at out partition 64).
                Odd heads (B): col 0 = ones (denominator at out partition 32),
                cols [1:32] zeros, cols [32:96] = v (out partitions 64:128).
                """
                vA = proj.tile([128, 8, 4, 65], BF16, name=out_tag + "A",
                               tag=out_tag + "A")
                vB = proj.tile([128, 8, 4, 64], BF16, name=out_tag + "B",
                               tag=out_tag + "B")
                nc.vector.memset(vA[:, :, :, 64:65], 1.0)
                with tc.tile_pool(name="pv", bufs=1, space="PSUM") as pvp:
                    for half in range(2):
                        wv_t = wpool.tile([128, 1, DT, 512], BF16, name="wv_t",
                                          tag="wbig")
                        nc.sync.dma_start(wv_t[:], w_dram[:, half:half + 1])
                        pvs = [pvp.tile([128, 512], F32, name="pv", tag="pv",
                                        bufs=4) for _ in range(4)]
                        for dt_i in range(DT):
                            for tt in range(4):
                                nc.tensor.matmul(
                                    pvs[tt][:],
                                    n_src[:, dt_i, tt * 128:(tt + 1) * 128],
                                    wv_t[:, 0, dt_i], start=(dt_i == 0),
                                    stop=(dt_i == DT - 1))
                        for tt in range(4):
                            src = pvs[tt].rearrange(
                                "p (g two e) -> p g two e", two=2, e=64)
                            g0 = half * 4
                            if tt % 2 == 0:
                                nc.scalar.copy(
                                    vA[:, g0:g0 + 4, tt, 0:64],
                                    src[:, :, 0])
                                nc.vector.tensor_copy(
                                    vB[:, g0:g0 + 4, tt, 0:64],
                                    src[:, :, 1])
                            else:
                                nc.vector.tensor_copy(
                                    vA[:, g0:g0 + 4, tt, 0:64],
                                    src[:, :, 0])
                                nc.scalar.copy(
                                    vB[:, g0:g0 + 4, tt, 0:64],
                                    src[:, :, 1])
                return vA, vB

            for l in range(NL):
                slot = l % 2
                ppt = small.tile([128, PP_W], F32, name="ppt", tag="ppt")
                nc.sync.dma_start(ppt[:], pp[l])
                lt = small.tile([1, 2, DT, 128], BF16, name="lt", tag="lt")
                nc.sync.dma_start(lt[:], lnlhs[l])

                n8 = peerp.tile([128, DT, 512], FP8, name="n8", tag="n8")
                mark('ln1')
                n1 = layernorm(xT, ppt[:, PP_S1:PP_S1 + 8], lt[:, 0],
                               fp8_out=n8)

                # ---- ship normed x (fp8); start pair AllGather
                nc.scalar.dma_start(
                    ag_in[slot].rearrange("(p a t) -> p a t", p=128, a=DT),
                    n8[:])
                nc.gpsimd.collective_compute(
                    "AllGather", mybir.AluOpType.bypass,
                    replica_groups=groups,
                    ins=[ag_in[slot][:]], outs=[ag_out[slot][:]])

                # ---- own-token projections
                mark('k_own')
                kt_o = proj_kt(n1, wkp[l], "kt_o")
                mark('v_own')
                vA_o, vB_o = proj_v(n1, wvp[l], "v_o")
                mark('q_own')
                qt = proj_kt(n1, wqp[l], "qt")

                # ---- attention
                oT_all = proj.tile([128, DT, 512], BF16, name="oT_all",
                                   tag="oT")
                locA = locp.tile([128, DT, 512], BF16, name="locA", tag="locA")
                locB = locp.tile([128, DT, 512], BF16, name="locB", tag="locB")

                def attn_pass(pat, t, kt_x, vA_x, vB_x, kind, psab_bufs):
                    """One causal pass for head pair (2t, 2t+1).

                    poA bank: rows [0:65) = head-A AV + its denominator
                    (row 64); row 96 = head-B denominator (rides the same
                    accumulation group via start/stop ordering)."""
                    poA = pat.tile([128, 512], F32, name="poA", tag="poA")
                    poB = pat.tile([128, 512], F32, name="poB", tag="poB")
                    dent = pat.tile([128, 512], F32, name="dent", tag="dent")
                    for j in range(4):
                        off = 128 * j
                        psAB = pat.tile([128, 2, 512], F32, name="psAB",
                                        tag="psAB", bufs=psab_bufs)
                        nc.tensor.matmul(psAB[:, 0, off:],
                                         kt_x[0:64, t, off:off + 128],
                                         qt[0:64, t, off:],
                                         start=True, stop=True)
                        nc.tensor.matmul(psAB[:, 1, off:],
                                         kt_x[64:128, t, off:off + 128],
                                         qt[64:128, t, off:],
                                         start=True, stop=True)
                        stAB = stp.tile([128, 2, 512], BF16, name="stAB",
                                        tag="stAB", bufs=4)
                        nc.scalar.activation(
                            stAB[:, :, off:], psAB[:, :, off:],
                            mybir.ActivationFunctionType.Exp,
                            scale=INV_SQRT_C)
                        nc.vector.tensor_mul(stAB[:, 0, off:off + 128],
                                             stAB[:, 0, off:off + 128],
                                             mask_t[:, kind + 1, 0])
                        nc.gpsimd.tensor_mul(stAB[:, 1, off:off + 128],
                                             stAB[:, 1, off:off + 128],
                                             mask_t[:, kind + 1, 1])
                        st, sp = (j == 0), (j == 3)
                        nc.tensor.matmul(poA[0:65, off:],
                                         vA_x[:, t, j, 0:65],
                                         stAB[:, 0, off:], start=st,
                                         stop=sp)
                        nc.tensor.matmul(poB[64:128, off:],
                                         vB_x[:, t, j, 0:64],
                                         stAB[:, 1, off:], start=st,
                                         stop=sp)
                        nc.tensor.matmul(dent[0:1, off:], ones_bf[:],
                                         stAB[:, 1, off:], start=st,
                                         stop=sp)
                    return poA, poB, dent

                mark('attn_loc')
                mark('attn_loc')
                with tc.tile_pool(name="patl", bufs=1, space="PSUM") as pat:
                    for t in range(DT):
                        poA, poB, dent = attn_pass(pat, t, kt_o, vA_o, vB_o,
                                                   0, 2)
                        if t % 2 == 0:
                            nc.scalar.copy(locA[0:65, t], poA[0:65])
                            nc.vector.tensor_copy(locB[64:128, t],
                                                  poB[64:128])
                        else:
                            nc.vector.tensor_copy(locA[0:65, t], poA[0:65])
                            nc.scalar.copy(locB[64:128, t], poB[64:128])
                        nc.vector.tensor_copy(locB[0:1, t], dent[0:1])

                # ---- peer block: gather x, project K/V locally
                n1p = peerp.tile([128, DT, 512], FP8, name="n1p",
                                 tag="n8")
                blk = ag_out[slot].rearrange("(b z) -> b z", b=2)[
                    bass.ds(peer, 1), :]
                nc.gpsimd.dma_start(
                    n1p[:], blk.rearrange("o (p a t) -> o p a t",
                                          p=128, a=DT))
                mark('k_peer')
                kt_p = proj_kt(n1p, wkp[l], "kt_p")
                mark('v_peer')
                vA_p, vB_p = proj_v(n1p, wvp[l], "v_p")

                mark('attn_rem')
                with tc.tile_pool(name="patr", bufs=1, space="PSUM") as pat:
                    for t in range(DT):
                        poA, poB, dent = attn_pass(pat, t, kt_p, vA_p, vB_p,
                                                   1, 2)
                        # combine + normalize
                        sumA = sump.tile([128, 512], F32, name="sumA",
                                         tag="sumA", bufs=2)
                        nc.vector.tensor_add(sumA[0:65], poA[0:65],
                                             locA[0:65, t])
                        recipA = stp.tile([1, 512], BF16, name="recipA",
                                          tag="recip", bufs=2)
                        with nc.allow_low_precision(reason="bf16 softmax den"):
                            nc.vector.reciprocal(recipA[:], sumA[64:65])
                        pbcA = pat.tile([128, 512], F32, name="pbcA",
                                        tag="pbc", bufs=1)
                        nc.tensor.matmul(pbcA[0:64], onesb[0:1, 0:64],
                                         recipA[:], start=True, stop=True)
                        nc.vector.tensor_mul(oT_all[0:64, t], sumA[0:64],
                                             pbcA[0:64])
                        sumB = sump.tile([128, 512], F32, name="sumB",
                                         tag="sumB", bufs=2)
                        nc.vector.tensor_add(sumB[64:128], poB[64:128],
                                             locB[64:128, t])
                        nc.vector.tensor_add(sumB[0:1], dent[0:1],
                                             locB[0:1, t])
                        recipB = stp.tile([1, 512], BF16, name="recipB",
                                          tag="recip", bufs=2)
                        with nc.allow_low_precision(reason="bf16 softmax den"):
                            nc.vector.reciprocal(recipB[:], sumB[0:1])
                        pbcB = pat.tile([128, 512], F32, name="pbcB",
                                        tag="pbc", bufs=1)
                        nc.tensor.matmul(pbcB[64:128], onesb[0:1, 0:64],
                                         recipB[:], start=True, stop=True)
                        nc.vector.tensor_mul(oT_all[64:128, t], sumB[64:128],
                                             pbcB[64:128])

                mark('wo')
                x2 = xpool.tile([128, DT, 512], F32, name="x2", tag="x")
                with tc.tile_pool(name="pwo", bufs=3, space="PSUM") as pwo:
                    for half in range(2):
                        wo_t = wpool.tile([128, 4, DT, 128], BF16, name="wo_t",
                                          tag="wbig")
                        nc.sync.dma_start(wo_t[:],
                                          wop[l, :, half * 4:(half + 1) * 4])
                        for di in range(4):
                            dto = half * 4 + di
                            pw = pwo.tile([128, 512], F32, name="pw", tag="pw")
                            for et in range(DT):
                                nc.tensor.matmul(pw[:], wo_t[:, di, et],
                                                 oT_all[:, et],
                                                 start=(et == 0),
                                                 stop=(et == DT - 1))
                            nc.vector.scalar_tensor_tensor(
                                x2[:, dto], pw[:],
                                ppt[:, PP_BO + dto:PP_BO + dto + 1],
                                n1[:, dto], mybir.AluOpType.add,
                                mybir.AluOpType.add)

                mark('ln2')
                n2 = layernorm(x2, ppt[:, PP_S2:PP_S2 + 8], lt[:, 1])

                # ---- MLP
                hT = proj.tile([128, 32, 512], BF16, name="hT", tag="hT")
                x3 = xpool.tile([128, DT, 512], F32, name="x3", tag="x")
                mark('mlp')
                with tc.tile_pool(name="pmlp", bufs=2, space="PSUM") as pmlp:
                    for hp in range(8):
                        w1_t = wpool.tile([128, 4, DT, 128], BF16, name="w1_t",
                                          tag="wbig")
                        nc.sync.dma_start(w1_t[:],
                                          w1p[l, :, hp * 4:(hp + 1) * 4])
                        for hi in range(4):
                            ht = hp * 4 + hi
                            ph = pmlp.tile([128, 512], F32, name="ph",
                                           tag="ph")
                            for dt_i in range(DT):
                                nc.tensor.matmul(ph[:], w1_t[:, hi, dt_i],
                                                 n2[:, dt_i],
                                                 start=(dt_i == 0),
                                                 stop=(dt_i == DT - 1))
                            bcol = ppt[:, PP_B1 + ht:PP_B1 + ht + 1]
                            if ht % 2 == 0:
                                nc.scalar.activation(
                                    hT[:, ht], ph[:],
                                    mybir.ActivationFunctionType.Relu,
                                    bias=bcol)
                            else:
                                nc.vector.tensor_scalar(
                                    hT[:, ht], ph[:], bcol, 0.0,
                                    mybir.AluOpType.add,
                                    mybir.AluOpType.max)
                    for dt_i in range(DT):
                        w2_t = wpool.tile([128, 1, 32, 128], BF16, name="w2_t",
                                          tag="w2", bufs=2)
                        nc.sync.dma_start(w2_t[:], w2p[l, :, dt_i:dt_i + 1])
                        py = pmlp.tile([128, 512], F32, name="py", tag="py")
                        for ht in range(32):
                            nc.tensor.matmul(py[:], w2_t[:, 0, ht], hT[:, ht],
                                             start=(ht == 0), stop=(ht == 31))
                        nc.vector.scalar_tensor_tensor(
                            x3[:, dt_i], py[:],
                            ppt[:, PP_B2 + dt_i:PP_B2 + dt_i + 1],
                            n2[:, dt_i], mybir.AluOpType.add,
                            mybir.AluOpType.add)
                xT = x3

        # ---- final LN + LM head
        mark('lmhead')
        nf = layernorm(xT, lnf_s_t[:], lnf_lhs_t[:, 0])
        out_r = out.rearrange("(q p) v -> p q v", p=128)
        with tc.tile_pool(name="lmw", bufs=3) as lmp, \
             tc.tile_pool(name="osb", bufs=3) as osb, \
             tc.tile_pool(name="plm", bufs=8, space="PSUM") as plm:
            for vg in range(NVG):
                lw = lmp.tile([128, DT, 512], BF16, name="lw", tag="lw")
                nc.sync.dma_start(lw[:], lmwp[vg])
                pls = [plm.tile([128, 512], F32, name="plm", tag="plm")
                       for _ in range(4)]
                for dt_i in range(DT):
                    for qi in range(4):
                        nc.tensor.matmul(
                            pls[qi][:],
                            nf[:, dt_i, qi * 128:(qi + 1) * 128],
                            lw[:, dt_i], start=(dt_i == 0),
                            stop=(dt_i == DT - 1))
                ob = osb.tile([128, 4, 512], BF16, name="ob", tag="ob")
                for qi in range(4):
                    if qi < 2:
                        nc.scalar.copy(ob[:, qi], pls[qi][:])
                    else:
                        nc.vector.tensor_copy(ob[:, qi], pls[qi][:])
                nc.scalar.dma_start(out_r[:, :, vg * 512:(vg + 1) * 512],
                                    ob[:])

    nc.compile()
    return nc


def host_prep(inputs, num_layers=L):
    """Per-core in_maps + reassembly metadata from full inputs."""
    f32 = np.float32
    bf = ml_dtypes.bfloat16
    idx = np.asarray(inputs["idx"])
    tok_emb = np.asarray(inputs["tok_emb"], f32)
    pos_emb = np.asarray(inputs["pos_emb"], f32)

    def perD(a):  # [L?, D] -> [L?, 128, DT]
        a = np.asarray(a, f32)
        if a.ndim == 1:
            return np.ascontiguousarray(a.reshape(DT, 128).T)
        return np.ascontiguousarray(
            a.reshape(a.shape[0], -1, 128).transpose(0, 2, 1))

    NLx = num_layers

    def panelK(w):  # [L, D, D] -> [L, 128(p), 8(ft), 8(dt), 128(f)]
        w = np.asarray(w, f32)[:NLx].astype(bf)
        return np.ascontiguousarray(
            w.reshape(NLx, DT, 128, DT, 128).transpose(0, 2, 3, 1, 4))

    wqp = panelK(inputs["Wq"])
    wkp = panelK(inputs["Wk"])
    wop = panelK(inputs["Wo"])
    wv = np.asarray(inputs["Wv"], f32)[:NLx].astype(bf)
    wvp = np.ascontiguousarray(
        wv.reshape(NLx, DT, 128, 2, 512).transpose(0, 2, 3, 1, 4))
    w1 = np.asarray(inputs["W1"], f32)[:NLx].astype(bf)
    w1p = np.ascontiguousarray(
        w1.reshape(NLx, DT, 128, 32, 128).transpose(0, 2, 3, 1, 4))
    w2 = np.asarray(inputs["W2"], f32)[:NLx].astype(bf)
    w2p = np.ascontiguousarray(
        w2.reshape(NLx, 32, 128, DT, 128).transpose(0, 2, 3, 1, 4))

    # packed per-layer params [NL, 128, PP_W]
    ppk = np.zeros((NLx, 128, PP_W), f32)
    ppk[:, :, PP_S1:PP_S1 + 8] = perD(inputs["ln1_s"])[:NLx]
    ppk[:, :, PP_S2:PP_S2 + 8] = perD(inputs["ln2_s"])[:NLx]
    ppk[:, :, PP_BO:PP_BO + 8] = perD(inputs["bo"])[:NLx]
    ppk[:, :, PP_B1:PP_B1 + 32] = perD(inputs["b1"])[:NLx]
    ppk[:, :, PP_B2:PP_B2 + 8] = perD(inputs["b2"])[:NLx]

    def negbs_rows(s, b):  # [L?, D] each -> [L?, DT, 128] of -b/s
        s = np.asarray(s, f32)
        b = np.asarray(b, f32)
        with np.errstate(divide="ignore", invalid="ignore"):
            r = np.where(s != 0, -b / np.where(s != 0, s, 1.0), 0.0)
        return r.reshape(r.shape[0], DT, 128) if r.ndim == 2 else \
            r.reshape(DT, 128)

    lnlhs = np.zeros((NLx, 1, 2, DT, 128), f32)
    lnlhs[:, 0, 0] = negbs_rows(inputs["ln1_s"][:NLx], inputs["ln1_b"][:NLx])
    lnlhs[:, 0, 1] = negbs_rows(inputs["ln2_s"][:NLx], inputs["ln2_b"][:NLx])
    lnlhs = lnlhs.astype(bf)

    lnfs = perD(inputs["lnf_s"])
    lnflhs = np.zeros((1, 2, DT, 128), f32)
    lnflhs[0, 0] = negbs_rows(np.asarray(inputs["lnf_s"])[None],
                              np.asarray(inputs["lnf_b"])[None])[0]
    lnflhs = lnflhs.astype(bf)

    lmw = np.zeros((D, VPAD), f32)
    lmw[:, :V] = np.asarray(inputs["lm_W"], f32)
    lmw = lmw.astype(bf)
    lmwp = np.ascontiguousarray(
        lmw.reshape(DT, 128, NVG, 512).transpose(2, 1, 0, 3))

    tri = np.tril(np.ones((128, 128), f32)).T  # mask[k, q] = 1 if k <= q
    m_ones = np.ones((128, 128), f32)
    m_zero = np.zeros((128, 128), f32)

    in_maps = []
    tiles_by_parity = []
    for c in range(8):
        b, p = c // 2, c % 2
        g_tiles = [2 * j + 1 - p for j in range(QT)]
        tiles_by_parity.append(g_tiles)
        rows = np.concatenate(
            [np.arange(g * 128, (g + 1) * 128) for g in g_tiles])
        x0 = tok_emb[idx[b, rows]] + pos_emb[rows]          # [512, D]
        x0T = np.ascontiguousarray(
            x0.T.reshape(DT, 128, 512).transpose(1, 0, 2)).astype(f32)
        mk = np.stack([m_zero, tri, m_ones if p == 0 else m_zero])
        mk = np.repeat(mk[:, :, None, :], 2, axis=2)         # [3, 128, 2, 128]
        in_maps.append(dict(
            x0T=x0T, wqp=wqp, wkp=wkp, wvp=wvp, wop=wop, w1p=w1p, w2p=w2p,
            pp=ppk, lnlhs=lnlhs, lnfs=lnfs, lnflhs=lnflhs,
            lmwp=lmwp, mident=mk.astype(bf),
            peer_i=np.array([[1 - p]], np.int32),
        ))
    return in_maps, tiles_by_parity


def assemble(results, tiles_by_parity, lm_b):
    out = np.empty((B, T, V), np.float32)
    lmb = np.asarray(lm_b, np.float32)[:V]
    for c in range(8):
        b = c // 2
        co = np.asarray(results[c]["out"], dtype=np.float32)
        for j, g in enumerate(tiles_by_parity[c]):
            out[b, g * 128:(g + 1) * 128] = co[j * 128:(j + 1) * 128, :V] + lmb
    return out


_CACHE = {}


def run(inputs, num_layers=L, trace=False):
    in_maps, tiles = host_prep(inputs, num_layers)
    key = num_layers
    if key not in _CACHE:
        _CACHE[key] = build_nc(num_layers)
    nc = _CACHE[key]
    res = run_bass_kernel_spmd(nc, in_maps, core_ids=list(range(8)),
                               trace=trace)
    return assemble(res.results, tiles, inputs["lm_b"]), res


def kernel(**inputs):
    out, _ = run(inputs, L)
    return out
